# revision 8
# baseline (speedup 1.0000x reference)
"""EntropicLayer (GCN conv + entropy gradient) on 8 trn2 NeuronCores.

v2 design: 3 heavy SPMD launches -> 2 (L1, L2) + 2 micro launches
(L2.5: c weights; L3: sparse correction), zero cross-core device traffic.

  L1: x16 (bf16) replicated -> per-core transposed-scatter over dst-owned
      edges+self-loops with full-product weights dinv_s*dinv_d folded into
      the one-hot selection -> aggT -> @W (f32) + b -> zT f32, z16 bf16,
      q = ||z16||^2 exact f32 with bf16 hi/lo split.
  L2: z16 replicated -> gather z16[src] by dst owner, unweighted 0/1 sel
      -> C (exact f32 sums of bf16), Sigma q_src via hi/lo bf16 matmul
      (exact to ~1e-6 rel) -> E exact-in-z16 -> softmax partials.
  L2.5: c_i = p_i(logp_i + S)/T from E + global partials (micro).
  L3: |c| >= eps nodes only (~20 nodes / ~50 edges for this regime):
      gather z16[dst] for selected edges routed by src owner, c-weighted
      sel -> B, A -> corrected output rows (micro).
Host between launches does ONLY data movement: concat, transpose, cast,
index-permute, threshold-routing. All value arithmetic stays on device.

Edge aggregation: edges sorted by dst tile; per 128-edge block one fused
DVE tensor_scalar builds the (weighted) one-hot selection [slot, col] in
bf16; PE scatters via bf16 matmul into fp32 PSUM. Gathers use dma_gather
(int16 indices split into A (idx < HALF) / B streams, 256B bf16 rows).
"""

import math
import numpy as np
import ml_dtypes

import concourse.bass as bass
import concourse.bacc as bacc
import concourse.mybir as mybir
import concourse.tile as tile

P = 128
D = 128
N_NODES = 50000
N_CORES = 8
F32 = mybir.dt.float32
BF16 = mybir.dt.bfloat16
I16 = mybir.dt.int16
AL = mybir.AluOpType
ACTF = mybir.ActivationFunctionType
AX = mybir.AxisListType
NPBF = ml_dtypes.bfloat16

EPS_C = 1e-8
CH = 1024  # idxs per dma_gather call
TB = 8     # tiles per gather batch


def cdiv(a, b):
    return (a + b - 1) // b


# ----------------------------------------------------------------------------
# Host-side layout prep (indices/structure only -- no input-value arithmetic)
# ----------------------------------------------------------------------------

class PassLayout:
    """Per-pass slot/block layout, uniform across cores."""

    def __init__(self, N, n_cores, TB=TB):
        self.N = N
        self.n_cores = n_cores
        self.R = N // n_cores
        self.NT = cdiv(self.R, P)
        self.RP = self.NT * P
        self.HALF = min(32768, (N + 1) // 2)
        assert self.HALF <= 32768 and N - self.HALF <= 32768
        self.TB = TB
        self.batches = []
        t = 0
        while t < self.NT:
            ntk = min(TB, self.NT - t)
            self.batches.append((t, ntk))
            t += ntk

    def build(self, gidx, cnode, lookup):
        N, n_cores, R, NT, HALF = self.N, self.n_cores, self.R, self.NT, self.HALF
        gidx = np.asarray(gidx, dtype=np.int64)
        cnode = np.asarray(cnode, dtype=np.int64)
        owner = cnode // R
        loc = cnode - owner * R
        tl = loc // P
        col = loc % P
        h = (gidx >= HALF).astype(np.int64)

        key = (owner * NT + tl) * 2 + h
        counts = np.bincount(key, minlength=n_cores * NT * 2).reshape(n_cores, NT, 2)
        BA_t = np.maximum(1, (counts[:, :, 0].max(axis=0) + P - 1) // P).astype(np.int64)
        BB_t = np.maximum(1, (counts[:, :, 1].max(axis=0) + P - 1) // P).astype(np.int64)
        self.BA_t, self.BB_t = BA_t, BB_t
        astart = np.zeros(NT + 1, dtype=np.int64)
        astart[1:] = np.cumsum(BA_t * P)
        bstart = np.zeros(NT + 1, dtype=np.int64)
        bstart[1:] = np.cumsum(BB_t * P)
        self.astart, self.bstart = astart, bstart
        self.SA = int(astart[-1])
        self.SB = int(bstart[-1])
        ABLK0 = np.zeros(NT, dtype=np.int64)
        BBLK0 = np.zeros(NT, dtype=np.int64)
        base_blk = []
        acc = 0
        maxb = 0
        for (t0, ntk) in self.batches:
            base_blk.append(acc)
            a = acc
            for j in range(ntk):
                ABLK0[t0 + j] = a
                a += int(BA_t[t0 + j])
            for j in range(ntk):
                BBLK0[t0 + j] = a
                a += int(BB_t[t0 + j])
            maxb = max(maxb, a - acc)
            acc = a
        self.ABLK0, self.BBLK0 = ABLK0, BBLK0
        self.base_blk = base_blk
        self.NBLK = acc
        self.MAXB = maxb

        order = np.argsort(key, kind="stable")
        ks = key[order]
        group_start = np.zeros(n_cores * NT * 2, dtype=np.int64)
        cnt_flat = np.bincount(key, minlength=n_cores * NT * 2)
        group_start[1:] = np.cumsum(cnt_flat)[:-1]
        pos = np.arange(len(order)) - group_start[ks]

        og, oc, ot, oh, ocol = (gidx[order], owner[order], tl[order], h[order], col[order])
        olook = lookup[order] if lookup is not None else None

        cores = []
        for c in range(n_cores):
            m = oc == c
            cg, ct, chh, ccol, cpos = og[m], ot[m], oh[m], ocol[m], pos[m]
            clook = olook[m] if olook is not None else None

            idxA = np.zeros(self.SA, dtype=np.int16)
            idxB = np.zeros(self.SB, dtype=np.int16)
            colid = np.full((self.NBLK, P), -1.0, dtype=np.float32)
            lookn = np.full((self.NBLK, P), -1, dtype=np.int64)

            for half, (idxarr, tstart, tblk0, off) in enumerate(
                [(idxA, astart, ABLK0, 0), (idxB, bstart, BBLK0, HALF)]
            ):
                mm = chh == half
                tt, ppos, gg, ccc = ct[mm], cpos[mm], cg[mm], ccol[mm]
                ll = clook[mm] if clook is not None else None
                stream = tstart[tt] + ppos
                idxarr[stream] = (gg - off).astype(np.int16)
                i = ppos // P
                p = ppos % P
                blk = tblk0[tt] + i
                colid[blk, p] = ccc.astype(np.float32)
                if ll is not None:
                    lookn[blk, p] = ll

            cores.append(
                dict(
                    idxA=wrap_idx(idxA),
                    idxB=wrap_idx(idxB),
                    colid=np.ascontiguousarray(colid.T),  # [P, NBLK] f32 (scalar operand)
                    lookn=lookn,  # [NBLK, P]
                )
            )
        self.cores = cores
        return self

    def permute(self, core, full_vec, padval, dtype=np.float32):
        """Per-slot values full_vec[lookn], padval where pad. -> [P, NBLK]"""
        lookn = self.cores[core]["lookn"]
        out = np.full(lookn.shape, padval, dtype=np.float32)
        m = lookn >= 0
        out[m] = np.asarray(full_vec, dtype=np.float32)[lookn[m]]
        return np.ascontiguousarray(out.T.astype(dtype))  # [P, NBLK]


def wrap_idx(flat):
    S = len(flat)
    assert S % 16 == 0
    arr = flat.reshape(S // 16, 16).T  # [16, S/16]
    return np.ascontiguousarray(np.tile(arr, (8, 1)))  # [128, S/16]


def host_prep(edge_index, N, n_cores):
    src = np.asarray(edge_index[0], dtype=np.int64)
    dst = np.asarray(edge_index[1], dtype=np.int64)
    E = len(src)

    loop = np.arange(N, dtype=np.int64)
    g1 = np.concatenate([src, loop])
    c1 = np.concatenate([dst, loop])
    deg = np.bincount(c1, minlength=N).astype(np.float64)
    dinv = deg ** -0.5
    wfull = (dinv[g1] * dinv[c1]).astype(np.float32)  # static edge weights
    indeg = np.bincount(dst, minlength=N).astype(np.float32)

    L1 = PassLayout(N, n_cores).build(g1, c1, lookup=np.arange(len(g1)))
    L2 = PassLayout(N, n_cores).build(src, dst, lookup=src)

    R = N // n_cores
    per_core = []
    for c in range(n_cores):
        lo = c * R
        RP = L1.RP
        indeg_pm = np.zeros((P, L1.NT), dtype=np.float32)
        mask_pm = np.zeros((P, L1.NT), dtype=np.float32)
        v = np.arange(RP)
        valid = v < R
        indeg_pm[v % P, v // P] = np.where(valid, indeg[np.minimum(lo + v, N - 1)], 0.0)
        mask_pm[v % P, v // P] = valid.astype(np.float32)
        w1 = L1.permute(c, wfull, 0.0)  # [P, NBLK1] f32 (scalar operand)
        per_core.append(dict(indeg_pm=indeg_pm, mask_pm=mask_pm, w1=w1))

    return dict(src=src, dst=dst, indeg=indeg, L1=L1, L2=L2,
                per_core=per_core, R=R, N=N)


# ----------------------------------------------------------------------------
# Device builders
# ----------------------------------------------------------------------------

IOTA_NP = np.tile(np.arange(P, dtype=np.float32), (P, 1))


def _gather_batches(nc, lay, gpool, x_dram, idxA_t, idxB_t):
    """Yield (k, t0, ntk, gtile) with both A and B regions gathered (bf16)."""
    HALF = lay.HALF
    N = lay.N
    aofs = 0
    bofs = 0
    for k, (t0, ntk) in enumerate(lay.batches):
        g = gpool.tile([P, lay.MAXB, D], BF16, tag="gat")
        nAblk = int(sum(lay.BA_t[t0: t0 + ntk]))
        nBblk = int(sum(lay.BB_t[t0: t0 + ntk]))
        nA = nAblk * P
        nB = nBblk * P
        for off in range(0, nA, CH):
            n = min(CH, nA - off)
            nc.gpsimd.dma_gather(
                g[:, off // P: (off + n) // P, :],
                x_dram[0:HALF, :],
                idxA_t[:, aofs + off // 16: aofs + (off + n) // 16],
                n, n, D, elem_step=D,
            )
        for off in range(0, nB, CH):
            n = min(CH, nB - off)
            nc.gpsimd.dma_gather(
                g[:, nAblk + off // P: nAblk + (off + n) // P, :],
                x_dram[HALF:N, :],
                idxB_t[:, bofs + off // 16: bofs + (off + n) // 16],
                n, n, D, elem_step=D,
            )
        aofs += nA // 16
        bofs += nB // 16
        yield k, t0, ntk, g


def _sel_blocks(nc, spool, colid_t, iota_t, w_t, blk0, BH):
    """Per-block fused one-hot build: sel[:, i*P:(i+1)*P] =
    (iota is_equal colid[:,blk]) [mult w[:,blk]], bf16 throughout."""
    sel = spool.tile([P, BH * P], BF16, tag="sel")
    for i in range(BH):
        blk = blk0 + i
        if w_t is None:
            nc.vector.tensor_scalar(
                out=sel[:, i * P: (i + 1) * P], in0=iota_t[:],
                scalar1=colid_t[:, blk: blk + 1], scalar2=None, op0=AL.is_equal,
            )
        else:
            nc.vector.tensor_scalar(
                out=sel[:, i * P: (i + 1) * P], in0=iota_t[:],
                scalar1=colid_t[:, blk: blk + 1], scalar2=w_t[:, blk: blk + 1],
                op0=AL.is_equal, op1=AL.mult,
            )
    return sel


def build_L1(lay1, N):
    nc = bacc.Bacc("TRN2", target_bir_lowering=False, debug=False,
                   dynamic_dma_scratch_size=65536)
    NT, RP, NBLK = lay1.NT, lay1.RP, lay1.NBLK
    SA16, SB16 = lay1.SA // 16, lay1.SB // 16

    x16 = nc.dram_tensor("x16", [N, D], BF16, kind="ExternalInput")
    Wm = nc.dram_tensor("W", [D, D], F32, kind="ExternalInput")
    bm = nc.dram_tensor("b", [D, 1], F32, kind="ExternalInput")
    idxA = nc.dram_tensor("idxA", [P, SA16], I16, kind="ExternalInput")
    idxB = nc.dram_tensor("idxB", [P, SB16], I16, kind="ExternalInput")
    colid = nc.dram_tensor("colid", [P, NBLK], F32, kind="ExternalInput")
    wsl = nc.dram_tensor("wsl", [P, NBLK], F32, kind="ExternalInput")
    iota_c = nc.inline_tensor(IOTA_NP, "iota_c")

    zT_out = nc.dram_tensor("zT", [P, RP], F32, kind="ExternalOutput")
    z16T_out = nc.dram_tensor("z16T", [P, RP], BF16, kind="ExternalOutput")
    q_out = nc.dram_tensor("q_pm", [P, NT], F32, kind="ExternalOutput")
    qhi_out = nc.dram_tensor("qhi_pm", [P, NT], BF16, kind="ExternalOutput")
    qlo_out = nc.dram_tensor("qlo_pm", [P, NT], BF16, kind="ExternalOutput")

    with tile.TileContext(nc) as tc:
        with (
            tc.tile_pool(name="const", bufs=1) as cpool,
            tc.tile_pool(name="gat", bufs=2) as gpool,
            tc.tile_pool(name="sel", bufs=3) as spool,
            tc.tile_pool(name="work", bufs=3) as wpool,
            tc.tile_pool(name="psA", bufs=2, space="PSUM") as psA_pool,
            tc.tile_pool(name="psZ", bufs=2, space="PSUM") as psZ_pool,
            tc.tile_pool(name="psQ", bufs=2, space="PSUM") as psQ_pool,
        ):
            iota_f = cpool.tile([P, P], F32)
            nc.sync.dma_start(iota_f[:], iota_c[:])
            iota_t = cpool.tile([P, P], BF16)
            nc.vector.tensor_copy(out=iota_t[:], in_=iota_f[:])
            W_t = cpool.tile([P, P], F32)
            nc.sync.dma_start(W_t[:], Wm[:])
            b_t = cpool.tile([P, 1], F32)
            nc.sync.dma_start(b_t[:], bm[:])
            idxA_t = cpool.tile([P, SA16], I16)
            nc.sync.dma_start(idxA_t[:], idxA[:])
            idxB_t = cpool.tile([P, SB16], I16)
            nc.sync.dma_start(idxB_t[:], idxB[:])
            colid_t = cpool.tile([P, NBLK], F32)
            nc.sync.dma_start(colid_t[:], colid[:])
            w_t = cpool.tile([P, NBLK], F32)
            nc.sync.dma_start(w_t[:], wsl[:])
            ones_f = cpool.tile([P, 1], F32)
            nc.vector.memset(ones_f[:], 1.0)
            q_s = cpool.tile([P, NT], F32)

            for k, t0, ntk, g in _gather_batches(nc, lay1, gpool, x16, idxA_t, idxB_t):
                for j in range(ntk):
                    t = t0 + j
                    BAt, BBt = int(lay1.BA_t[t]), int(lay1.BB_t[t])
                    base = lay1.base_blk[k]
                    bufA = int(lay1.ABLK0[t]) - base
                    bufB = int(lay1.BBLK0[t]) - base
                    selA = _sel_blocks(nc, spool, colid_t, iota_t, w_t,
                                       int(lay1.ABLK0[t]), BAt)
                    selB = _sel_blocks(nc, spool, colid_t, iota_t, w_t,
                                       int(lay1.BBLK0[t]), BBt)
                    psAT = psA_pool.tile([P, P], F32, tag="aggT")
                    nblks = BAt + BBt
                    for bi in range(nblks):
                        if bi < BAt:
                            sl = selA[:, bi * P: (bi + 1) * P]
                            buf = bufA + bi
                        else:
                            i = bi - BAt
                            sl = selB[:, i * P: (i + 1) * P]
                            buf = bufB + i
                        # transposed scatter: out[feat, dstcol]
                        nc.tensor.matmul(
                            psAT[:], lhsT=g[:, buf, :], rhs=sl,
                            start=(bi == 0), stop=(bi == nblks - 1),
                        )
                    aggT_s = wpool.tile([P, P], F32, tag="aggTs")
                    nc.scalar.copy(out=aggT_s[:], in_=psAT[:])
                    psZ = psZ_pool.tile([P, P], F32, tag="zT")
                    nc.tensor.matmul(psZ[:], lhsT=W_t[:], rhs=aggT_s[:],
                                     start=True, stop=True)
                    zT_s = wpool.tile([P, P], F32, tag="zTs")
                    nc.scalar.activation(zT_s[:], psZ[:], ACTF.Identity, bias=b_t[:])
                    nc.sync.dma_start(zT_out[:, t * P: (t + 1) * P], zT_s[:])
                    z16_s = wpool.tile([P, P], BF16, tag="z16s")
                    nc.vector.tensor_copy(out=z16_s[:], in_=zT_s[:])
                    nc.sync.dma_start(z16T_out[:, t * P: (t + 1) * P], z16_s[:])
                    sq = wpool.tile([P, P], F32, tag="sq")
                    nc.vector.tensor_tensor(out=sq[:], in0=z16_s[:], in1=z16_s[:],
                                            op=AL.mult)
                    psQt = psQ_pool.tile([P, 8], F32, tag="q")
                    nc.tensor.matmul(psQt[:, 0:1], lhsT=sq[:], rhs=ones_f[:],
                                     start=True, stop=True)
                    nc.vector.tensor_copy(out=q_s[:, t: t + 1], in_=psQt[:, 0:1])
            qhi_s = wpool.tile([P, NT], BF16, tag="qhs")
            nc.vector.tensor_copy(out=qhi_s[:], in_=q_s[:])
            qhiF = wpool.tile([P, NT], F32, tag="qhF")
            nc.vector.tensor_copy(out=qhiF[:], in_=qhi_s[:])
            qloF = wpool.tile([P, NT], F32, tag="qlF")
            nc.vector.tensor_tensor(out=qloF[:], in0=q_s[:], in1=qhiF[:],
                                    op=AL.subtract)
            qlo_s = wpool.tile([P, NT], BF16, tag="qls")
            nc.vector.tensor_copy(out=qlo_s[:], in_=qloF[:])
            nc.sync.dma_start(q_out[:], q_s[:])
            nc.sync.dma_start(qhi_out[:], qhi_s[:])
            nc.sync.dma_start(qlo_out[:], qlo_s[:])
    nc.compile()
    return nc


def build_L2(lay2, N):
    nc = bacc.Bacc("TRN2", target_bir_lowering=False, debug=False,
                   dynamic_dma_scratch_size=65536)
    NT, RP, NBLK = lay2.NT, lay2.RP, lay2.NBLK
    SA16, SB16 = lay2.SA // 16, lay2.SB // 16

    z16 = nc.dram_tensor("z16", [N, D], BF16, kind="ExternalInput")
    zown = nc.dram_tensor("zown", [RP, D], F32, kind="ExternalInput")
    idxA = nc.dram_tensor("idxA", [P, SA16], I16, kind="ExternalInput")
    idxB = nc.dram_tensor("idxB", [P, SB16], I16, kind="ExternalInput")
    colid = nc.dram_tensor("colid", [P, NBLK], F32, kind="ExternalInput")
    qsl2 = nc.dram_tensor("qsl2", [P, 2 * NBLK], BF16, kind="ExternalInput")
    qown = nc.dram_tensor("qown", [P, NT], F32, kind="ExternalInput")
    indeg_pm = nc.dram_tensor("indeg_pm", [P, NT], F32, kind="ExternalInput")
    mask_pm = nc.dram_tensor("mask_pm", [P, NT], F32, kind="ExternalInput")
    Tcol = nc.dram_tensor("Tcol", [P, 1], F32, kind="ExternalInput")
    iota_c = nc.inline_tensor(IOTA_NP, "iota_c")

    C_out = nc.dram_tensor("C16", [RP, D], BF16, kind="ExternalOutput")
    E_out = nc.dram_tensor("E", [P, NT], F32, kind="ExternalOutput")
    s_out = nc.dram_tensor("spart", [1, 2], F32, kind="ExternalOutput")

    with tile.TileContext(nc) as tc:
        with (
            tc.tile_pool(name="const", bufs=1) as cpool,
            tc.tile_pool(name="gat", bufs=2) as gpool,
            tc.tile_pool(name="sel", bufs=3) as spool,
            tc.tile_pool(name="work", bufs=3) as wpool,
            tc.tile_pool(name="acc", bufs=1) as apool,
            tc.tile_pool(name="ps2", bufs=3, space="PSUM") as ps_pool,
            tc.tile_pool(name="psq", bufs=3, space="PSUM") as psq_pool,
            tc.tile_pool(name="psS", bufs=1, space="PSUM") as psS_pool,
        ):
            iota_f = cpool.tile([P, P], F32)
            nc.sync.dma_start(iota_f[:], iota_c[:])
            iota_t = cpool.tile([P, P], BF16)
            nc.vector.tensor_copy(out=iota_t[:], in_=iota_f[:])
            idxA_t = cpool.tile([P, SA16], I16)
            nc.sync.dma_start(idxA_t[:], idxA[:])
            idxB_t = cpool.tile([P, SB16], I16)
            nc.sync.dma_start(idxB_t[:], idxB[:])
            colid_t = cpool.tile([P, NBLK], F32)
            nc.sync.dma_start(colid_t[:], colid[:])
            q_t = cpool.tile([P, 2 * NBLK], BF16)
            nc.sync.dma_start(q_t[:], qsl2[:])
            qown_t = cpool.tile([P, NT], F32)
            nc.sync.dma_start(qown_t[:], qown[:])
            indeg_t = cpool.tile([P, NT], F32)
            nc.sync.dma_start(indeg_t[:], indeg_pm[:])
            mask_t = cpool.tile([P, NT], F32)
            nc.sync.dma_start(mask_t[:], mask_pm[:])
            T_t = cpool.tile([P, 1], F32)
            nc.sync.dma_start(T_t[:], Tcol[:])
            ones_f = cpool.tile([P, 1], F32)
            nc.vector.memset(ones_f[:], 1.0)
            E_sb = apool.tile([P, NT], F32)

            for k, t0, ntk, g in _gather_batches(nc, lay2, gpool, z16, idxA_t, idxB_t):
                for j in range(ntk):
                    t = t0 + j
                    BAt, BBt = int(lay2.BA_t[t]), int(lay2.BB_t[t])
                    base = lay2.base_blk[k]
                    bufA = int(lay2.ABLK0[t]) - base
                    bufB = int(lay2.BBLK0[t]) - base
                    selA = _sel_blocks(nc, spool, colid_t, iota_t, None,
                                       int(lay2.ABLK0[t]), BAt)
                    selB = _sel_blocks(nc, spool, colid_t, iota_t, None,
                                       int(lay2.BBLK0[t]), BBt)
                    psC = ps_pool.tile([P, D], F32, tag="C")
                    psq = psq_pool.tile([P, 8], F32, tag="qq")
                    nblks = BAt + BBt
                    for bi in range(nblks):
                        if bi < BAt:
                            sl = selA[:, bi * P: (bi + 1) * P]
                            buf = bufA + bi
                            blk = int(lay2.ABLK0[t]) + bi
                        else:
                            i = bi - BAt
                            sl = selB[:, i * P: (i + 1) * P]
                            buf = bufB + i
                            blk = int(lay2.BBLK0[t]) + i
                        nc.tensor.matmul(
                            psC[:, 0:D], lhsT=sl, rhs=g[:, buf, :],
                            start=(bi == 0), stop=(bi == nblks - 1),
                        )
                        nc.tensor.matmul(
                            psq[:, 0:2], lhsT=sl, rhs=q_t[:, 2 * blk: 2 * blk + 2],
                            start=(bi == 0), stop=(bi == nblks - 1),
                        )
                    C_s = wpool.tile([P, P], F32, tag="Cs")
                    nc.vector.tensor_copy(out=C_s[:], in_=psC[:, 0:D])
                    C16_s = wpool.tile([P, P], BF16, tag="C16s")
                    nc.scalar.copy(out=C16_s[:], in_=C_s[:])
                    nc.sync.dma_start(C_out[t * P: (t + 1) * P, :], C16_s[:])
                    z_t = wpool.tile([P, P], F32, tag="zt")
                    nc.sync.dma_start(z_t[:], zown[t * P: (t + 1) * P, :])
                    zc = wpool.tile([P, P], F32, tag="zc")
                    nc.vector.tensor_tensor(out=zc[:], in0=z_t[:], in1=C_s[:], op=AL.mult)
                    rd = wpool.tile([P, 1], F32, tag="rd")
                    nc.vector.reduce_sum(rd[:], zc[:], axis=AX.X)
                    # E = (psq0+psq1) + indeg*qown - 2*rd
                    psq_s = wpool.tile([P, 2], F32, tag="psqs")
                    nc.vector.tensor_copy(out=psq_s[:], in_=psq[:, 0:2])
                    sq2 = wpool.tile([P, 1], F32, tag="sq2")
                    nc.vector.tensor_tensor(out=sq2[:], in0=psq_s[:, 0:1],
                                            in1=psq_s[:, 1:2], op=AL.add)
                    t1 = wpool.tile([P, 1], F32, tag="t1")
                    nc.vector.tensor_tensor(
                        out=t1[:], in0=indeg_t[:, t: t + 1], in1=qown_t[:, t: t + 1],
                        op=AL.mult,
                    )
                    t2 = wpool.tile([P, 1], F32, tag="t2")
                    nc.vector.tensor_scalar(
                        out=t2[:], in0=rd[:], scalar1=-2.0, scalar2=None, op0=AL.mult
                    )
                    t3 = wpool.tile([P, 1], F32, tag="t3")
                    nc.vector.tensor_tensor(out=t3[:], in0=t2[:], in1=t1[:], op=AL.add)
                    nc.vector.tensor_tensor(
                        out=E_sb[:, t: t + 1], in0=t3[:], in1=sq2[:], op=AL.add
                    )
            # softmax partials
            Tinv = wpool.tile([P, 1], F32, tag="tinv")
            nc.vector.reciprocal(Tinv[:], T_t[:])
            negTinv = wpool.tile([P, 1], F32, tag="ntinv")
            nc.vector.tensor_scalar(
                out=negTinv[:], in0=Tinv[:], scalar1=-1.0, scalar2=None, op0=AL.mult
            )
            a_sb = wpool.tile([P, NT], F32, tag="asb")
            nc.vector.tensor_scalar(
                out=a_sb[:], in0=E_sb[:], scalar1=negTinv[:], scalar2=None, op0=AL.mult
            )
            expa = wpool.tile([P, NT], F32, tag="expa")
            nc.scalar.activation(expa[:], a_sb[:], ACTF.Exp)
            nc.vector.tensor_tensor(out=expa[:], in0=expa[:], in1=mask_t[:], op=AL.mult)
            s12 = wpool.tile([P, 2], F32, tag="s12")
            nc.vector.reduce_sum(s12[:, 0:1], expa[:], axis=AX.X)
            ea = wpool.tile([P, NT], F32, tag="ea")
            nc.vector.tensor_tensor(out=ea[:], in0=expa[:], in1=a_sb[:], op=AL.mult)
            nc.vector.reduce_sum(s12[:, 1:2], ea[:], axis=AX.X)
            psS = psS_pool.tile([1, 2], F32)
            nc.tensor.matmul(psS[:], lhsT=ones_f[:], rhs=s12[:], start=True, stop=True)
            sS = wpool.tile([1, 2], F32, tag="ss")
            nc.vector.tensor_copy(out=sS[:], in_=psS[:])
            nc.sync.dma_start(s_out[:], sS[:])
            nc.sync.dma_start(E_out[:], E_sb[:])
    nc.compile()
    return nc


def build_L25(NT, n_cores):
    """c_i = exp(a)*(a - s2/s1)/(s1*T), a = -E/T; masked on pads."""
    nc = bacc.Bacc("TRN2", target_bir_lowering=False, debug=False)
    E_in = nc.dram_tensor("E", [P, NT], F32, kind="ExternalInput")
    mask_pm = nc.dram_tensor("mask_pm", [P, NT], F32, kind="ExternalInput")
    Tcol = nc.dram_tensor("Tcol", [P, 1], F32, kind="ExternalInput")
    spart = nc.dram_tensor("spart", [1, 2 * n_cores], F32, kind="ExternalInput")
    c_out = nc.dram_tensor("c_pm", [P, NT], F32, kind="ExternalOutput")

    with tile.TileContext(nc) as tc:
        with (
            tc.tile_pool(name="work", bufs=1) as wpool,
            tc.tile_pool(name="psB", bufs=1, space="PSUM") as psB_pool,
        ):
            E_t = wpool.tile([P, NT], F32, tag="E")
            nc.sync.dma_start(E_t[:], E_in[:])
            mask_t = wpool.tile([P, NT], F32, tag="mask")
            nc.sync.dma_start(mask_t[:], mask_pm[:])
            T_t = wpool.tile([P, 1], F32, tag="T")
            nc.sync.dma_start(T_t[:], Tcol[:])
            sp_t = wpool.tile([1, 2 * n_cores], F32, tag="sp")
            nc.sync.dma_start(sp_t[:], spart[:])
            onesrow = wpool.tile([1, P], F32, tag="or")
            nc.vector.memset(onesrow[:], 1.0)

            s1 = wpool.tile([1, 1], F32, tag="s1")
            nc.vector.reduce_sum(s1[:], sp_t[0:1, 0:n_cores], axis=AX.X)
            s2 = wpool.tile([1, 1], F32, tag="s2")
            nc.vector.reduce_sum(s2[:], sp_t[0:1, n_cores: 2 * n_cores], axis=AX.X)
            r1 = wpool.tile([1, 1], F32, tag="r1")
            nc.vector.reciprocal(r1[:], s1[:])
            rr = wpool.tile([1, 1], F32, tag="rr")
            nc.vector.tensor_tensor(out=rr[:], in0=s2[:], in1=r1[:], op=AL.mult)
            Ti1 = wpool.tile([1, 1], F32, tag="ti1")
            nc.vector.reciprocal(Ti1[:], T_t[0:1, :])
            sc = wpool.tile([1, 1], F32, tag="sc")
            nc.vector.tensor_tensor(out=sc[:], in0=r1[:], in1=Ti1[:], op=AL.mult)
            sc2 = wpool.tile([1, 2], F32, tag="sc2")
            nc.vector.tensor_copy(out=sc2[:, 0:1], in_=rr[:])
            nc.vector.tensor_copy(out=sc2[:, 1:2], in_=sc[:])
            psB = psB_pool.tile([P, 2], F32)
            nc.tensor.matmul(psB[:], lhsT=onesrow[:], rhs=sc2[:], start=True, stop=True)
            bc = wpool.tile([P, 2], F32, tag="bc")
            nc.vector.tensor_copy(out=bc[:], in_=psB[:])
            Tinv = wpool.tile([P, 1], F32, tag="tinv")
            nc.vector.reciprocal(Tinv[:], T_t[:])
            negTinv = wpool.tile([P, 1], F32, tag="ntv")
            nc.vector.tensor_scalar(
                out=negTinv[:], in0=Tinv[:], scalar1=-1.0, scalar2=None, op0=AL.mult
            )
            a = wpool.tile([P, NT], F32, tag="a")
            nc.vector.tensor_scalar(
                out=a[:], in0=E_t[:], scalar1=negTinv[:], scalar2=None, op0=AL.mult
            )
            ex = wpool.tile([P, NT], F32, tag="ex")
            nc.scalar.activation(ex[:], a[:], ACTF.Exp)
            nc.vector.tensor_scalar(
                out=a[:], in0=a[:], scalar1=bc[:, 0:1], scalar2=None, op0=AL.subtract
            )
            nc.vector.tensor_tensor(out=ex[:], in0=ex[:], in1=a[:], op=AL.mult)
            nc.vector.tensor_scalar(
                out=ex[:], in0=ex[:], scalar1=bc[:, 1:2], scalar2=None, op0=AL.mult
            )
            nc.vector.tensor_tensor(out=ex[:], in0=ex[:], in1=mask_t[:], op=AL.mult)
            nc.sync.dma_start(c_out[:], ex[:])
    nc.compile()
    return nc


def build_L3(N, HALF, KP, BA3, BB3):
    """Sparse correction: out_k = z_k*(1 + 2w(A_k + c_k indeg_k))
    - 2w B_k - 2w c_k C_k for KP padded correction nodes."""
    nc = bacc.Bacc("TRN2", target_bir_lowering=False, debug=False,
                   dynamic_dma_scratch_size=65536)
    SA3, SB3 = BA3 * P, BB3 * P
    NBLK3 = BA3 + BB3

    z16 = nc.dram_tensor("z16", [N, D], BF16, kind="ExternalInput")
    zk = nc.dram_tensor("zk", [KP, D], F32, kind="ExternalInput")
    Ck = nc.dram_tensor("Ck", [KP, D], F32, kind="ExternalInput")
    ck = nc.dram_tensor("ck", [KP, 1], F32, kind="ExternalInput")
    indegk = nc.dram_tensor("indegk", [KP, 1], F32, kind="ExternalInput")
    wcol = nc.dram_tensor("wcol", [KP, 1], F32, kind="ExternalInput")
    idxA = nc.dram_tensor("idxA", [P, SA3 // 16], I16, kind="ExternalInput")
    idxB = nc.dram_tensor("idxB", [P, SB3 // 16], I16, kind="ExternalInput")
    colid = nc.dram_tensor("colid", [P, NBLK3], F32, kind="ExternalInput")
    csl = nc.dram_tensor("csl", [P, NBLK3], F32, kind="ExternalInput")
    iota_c = nc.inline_tensor(IOTA_NP, "iota_c")

    out = nc.dram_tensor("outk", [KP, D], F32, kind="ExternalOutput")

    with tile.TileContext(nc) as tc:
        with (
            tc.tile_pool(name="work", bufs=1) as wpool,
            tc.tile_pool(name="ps3", bufs=1, space="PSUM") as ps_pool,
        ):
            iota_f = wpool.tile([P, P], F32, tag="iof")
            nc.sync.dma_start(iota_f[:], iota_c[:])
            iota_t = wpool.tile([P, P], BF16, tag="io")
            nc.vector.tensor_copy(out=iota_t[:], in_=iota_f[:])
            colid_t = wpool.tile([P, NBLK3], F32, tag="col")
            nc.sync.dma_start(colid_t[:], colid[:])
            csl_t = wpool.tile([P, NBLK3], F32, tag="csl")
            nc.sync.dma_start(csl_t[:], csl[:])
            idxA_t = wpool.tile([P, SA3 // 16], I16, tag="ia")
            nc.sync.dma_start(idxA_t[:], idxA[:])
            idxB_t = wpool.tile([P, SB3 // 16], I16, tag="ib")
            nc.sync.dma_start(idxB_t[:], idxB[:])
            ones16 = wpool.tile([P, 1], BF16, tag="o16")
            nc.vector.memset(ones16[:], 1.0)
            g = wpool.tile([P, NBLK3, D], BF16, tag="g")
            nc.gpsimd.dma_gather(g[:, 0:BA3, :], z16[0:HALF, :], idxA_t[:],
                                 SA3, SA3, D, elem_step=D)
            nc.gpsimd.dma_gather(g[:, BA3:NBLK3, :], z16[HALF:N, :], idxB_t[:],
                                 SB3, SB3, D, elem_step=D)
            assert KP == P
            zk_t = wpool.tile([KP, D], F32, tag="zk")
            nc.sync.dma_start(zk_t[:], zk[:])
            Ck_t = wpool.tile([KP, D], F32, tag="Ckt")
            nc.sync.dma_start(Ck_t[:], Ck[:])
            ck_t = wpool.tile([KP, 1], F32, tag="ck")
            nc.sync.dma_start(ck_t[:], ck[:])
            ind_t = wpool.tile([KP, 1], F32, tag="ind")
            nc.sync.dma_start(ind_t[:], indegk[:])
            w_t = wpool.tile([KP, 1], F32, tag="w")
            nc.sync.dma_start(w_t[:], wcol[:])
            w2 = wpool.tile([KP, 1], F32, tag="w2")
            nc.vector.tensor_scalar(out=w2[:], in0=w_t[:], scalar1=2.0,
                                    scalar2=None, op0=AL.mult)

            sel = wpool.tile([P, NBLK3 * P], BF16, tag="sel")
            for i in range(NBLK3):
                nc.vector.tensor_scalar(
                    out=sel[:, i * P: (i + 1) * P], in0=iota_t[:],
                    scalar1=colid_t[:, i: i + 1], scalar2=csl_t[:, i: i + 1],
                    op0=AL.is_equal, op1=AL.mult,
                )
            psB = ps_pool.tile([KP, D], F32, tag="B")
            psA1 = ps_pool.tile([KP, 8], F32, tag="A")
            for i in range(NBLK3):
                sl = sel[:, i * P: (i + 1) * P]
                nc.tensor.matmul(psB[:], lhsT=sl, rhs=g[:, i, :],
                                 start=(i == 0), stop=(i == NBLK3 - 1))
                nc.tensor.matmul(psA1[:, 0:1], lhsT=sl, rhs=ones16[:],
                                 start=(i == 0), stop=(i == NBLK3 - 1))
            # u = A + c*indeg ; fac = 1 + 2w*u
            u = wpool.tile([KP, 1], F32, tag="u")
            nc.vector.tensor_tensor(out=u[:], in0=ck_t[:], in1=ind_t[:], op=AL.mult)
            nc.vector.tensor_tensor(out=u[:], in0=u[:], in1=psA1[:, 0:1], op=AL.add)
            fac = wpool.tile([KP, 1], F32, tag="fac")
            nc.vector.tensor_scalar(out=fac[:], in0=u[:], scalar1=w2[:],
                                    scalar2=1.0, op0=AL.mult, op1=AL.add)
            o = wpool.tile([KP, D], F32, tag="o")
            nc.vector.tensor_scalar(out=o[:], in0=zk_t[:], scalar1=fac[:],
                                    scalar2=None, op0=AL.mult)
            t1 = wpool.tile([KP, D], F32, tag="t1")
            nc.vector.tensor_scalar(out=t1[:], in0=psB[:], scalar1=w2[:],
                                    scalar2=None, op0=AL.mult)
            nc.vector.tensor_tensor(out=o[:], in0=o[:], in1=t1[:], op=AL.subtract)
            cw = wpool.tile([KP, 1], F32, tag="cw")
            nc.vector.tensor_tensor(out=cw[:], in0=ck_t[:], in1=w2[:], op=AL.mult)
            t2 = wpool.tile([KP, D], F32, tag="t2")
            nc.vector.tensor_scalar(out=t2[:], in0=Ck_t[:], scalar1=cw[:],
                                    scalar2=None, op0=AL.mult)
            nc.vector.tensor_tensor(out=o[:], in0=o[:], in1=t2[:], op=AL.subtract)
            nc.sync.dma_start(out[:], o[:])
    nc.compile()
    return nc


# ----------------------------------------------------------------------------
# Runtime driver
# ----------------------------------------------------------------------------

def full_pipeline(inputs_np, runner, n_cores=N_CORES):
    N = inputs_np["x"].shape[0]
    prep = host_prep(inputs_np["edge_index"], N, n_cores)
    R = prep["R"]
    lay1, lay2 = prep["L1"], prep["L2"]
    NT, RP = lay1.NT, lay1.RP
    x = np.asarray(inputs_np["x"], dtype=np.float32)
    W = np.ascontiguousarray(np.asarray(inputs_np["W"], dtype=np.float32))
    bcol = np.ascontiguousarray(
        np.asarray(inputs_np["b"], dtype=np.float32).reshape(D, 1))
    Tv = float(np.asarray(inputs_np["temperature"]).reshape(-1)[0])
    wv = float(np.asarray(inputs_np["weight"]).reshape(-1)[0])
    x16 = np.ascontiguousarray(x.astype(NPBF))

    # ---- L1 ----
    nc1 = build_L1(lay1, N)
    maps1 = []
    for c in range(n_cores):
        lc = lay1.cores[c]
        pc = prep["per_core"][c]
        maps1.append(dict(
            x16=x16, W=W, b=bcol,
            idxA=lc["idxA"], idxB=lc["idxB"], colid=lc["colid"], wsl=pc["w1"],
        ))
    res1 = runner(nc1, maps1, ["zT", "z16T", "q_pm", "qhi_pm", "qlo_pm"])

    zfull = np.concatenate(
        [np.asarray(res1[c]["zT"], dtype=np.float32).T[:R] for c in range(n_cores)],
        axis=0)
    z16full = np.ascontiguousarray(np.concatenate(
        [np.asarray(res1[c]["z16T"]).T[:R] for c in range(n_cores)], axis=0
    ).astype(NPBF))
    qfull = np.concatenate(
        [np.asarray(res1[c]["q_pm"], dtype=np.float32).T.reshape(-1)[:R]
         for c in range(n_cores)])
    qhifull = np.concatenate(
        [np.asarray(res1[c]["qhi_pm"]).T.reshape(-1)[:R] for c in range(n_cores)])
    qlofull = np.concatenate(
        [np.asarray(res1[c]["qlo_pm"]).T.reshape(-1)[:R] for c in range(n_cores)])

    # ---- L2 ----
    nc2 = build_L2(lay2, N)
    maps2 = []
    for c in range(n_cores):
        lc = lay2.cores[c]
        pc = prep["per_core"][c]
        lo = c * R
        zo = np.zeros((RP, D), dtype=np.float32)
        zo[:R] = z16full[lo: lo + R].astype(np.float32)  # exact cast-up
        qhi_sl = lay2.permute(c, qhifull, 0.0, dtype=NPBF)
        qlo_sl = lay2.permute(c, qlofull, 0.0, dtype=NPBF)
        qsl2 = np.empty((P, 2 * lay2.NBLK), dtype=NPBF)
        qsl2[:, 0::2] = qhi_sl
        qsl2[:, 1::2] = qlo_sl
        qown = np.zeros((P, NT), dtype=np.float32)
        v = np.arange(RP)
        qown[v % P, v // P] = np.where(v < R, qfull[np.minimum(lo + v, N - 1)], 0.0)
        maps2.append(dict(
            z16=z16full, zown=zo,
            idxA=lc["idxA"], idxB=lc["idxB"], colid=lc["colid"],
            qsl2=np.ascontiguousarray(qsl2), qown=qown,
            indeg_pm=pc["indeg_pm"], mask_pm=pc["mask_pm"],
            Tcol=np.full((P, 1), Tv, dtype=np.float32),
        ))
    res2 = runner(nc2, maps2, ["C16", "E", "spart"])

    s1s = [float(np.asarray(res2[c]["spart"])[0, 0]) for c in range(n_cores)]
    s2s = [float(np.asarray(res2[c]["spart"])[0, 1]) for c in range(n_cores)]
    sparts = np.array([s1s + s2s], dtype=np.float32)
    C16full = np.concatenate(
        [np.asarray(res2[c]["C16"])[:R] for c in range(n_cores)], axis=0)

    # ---- L2.5: c weights ----
    nc25 = build_L25(NT, n_cores)
    maps25 = []
    for c in range(n_cores):
        pc = prep["per_core"][c]
        maps25.append(dict(
            E=np.ascontiguousarray(np.asarray(res2[c]["E"], dtype=np.float32)),
            mask_pm=pc["mask_pm"],
            Tcol=np.full((P, 1), Tv, dtype=np.float32),
            spart=sparts,
        ))
    res25 = runner(nc25, maps25, ["c_pm"])
    cfull = np.concatenate(
        [np.asarray(res25[c]["c_pm"], dtype=np.float32).T.reshape(-1)[:R]
         for c in range(n_cores)])

    # ---- L3: sparse correction (host does routing/selection only) ----
    src, dst, indeg = prep["src"], prep["dst"], prep["indeg"]
    selN = np.abs(cfull) >= EPS_C
    selE = selN[dst]
    out = np.ascontiguousarray(zfull.astype(np.float32))
    knodes = np.union1d(np.where(selN)[0], np.unique(src[selE]))
    if len(knodes) > 0:
        res3, percore_nodes = run_L3_sparse(
            prep, runner, knodes, selE, cfull, z16full, zfull, C16full,
            indeg, Tv, wv, N)
        for c in range(n_cores):
            kn = percore_nodes[c]
            if len(kn):
                out[kn] = np.asarray(res3[c]["outk"], dtype=np.float32)[:len(kn)]
    return out


def run_L3_sparse(prep, runner, knodes, selE, cfull, z16full, zfull, C16full,
                  indeg, Tv, wv, N):
    n_cores = N_CORES
    R = prep["R"]
    src, dst = prep["src"], prep["dst"]
    HALF = prep["L1"].HALF
    KP = P
    esrc = src[selE]
    edst = dst[selE]

    percore_nodes = []
    percore = []
    maxA = maxB = 1
    for c in range(n_cores):
        lo, hi = c * R, (c + 1) * R
        kn = knodes[(knodes >= lo) & (knodes < hi)]
        assert len(kn) <= KP, f"correction set {len(kn)} > {KP} on core {c}"
        percore_nodes.append(kn)
        m = (esrc >= lo) & (esrc < hi)
        es, ed = esrc[m], edst[m]
        colpos = np.searchsorted(kn, es)
        hh = (ed >= HALF).astype(np.int64)
        nA = int((hh == 0).sum())
        nB = int((hh == 1).sum())
        maxA = max(maxA, cdiv(max(nA, 1), P))
        maxB = max(maxB, cdiv(max(nB, 1), P))
        percore.append((kn, es, ed, colpos, hh))
    BA3, BB3 = maxA, maxB
    NBLK3 = BA3 + BB3

    nc3 = build_L3(N, HALF, KP, BA3, BB3)
    maps3 = []
    for c in range(n_cores):
        kn, es, ed, colpos, hh = percore[c]
        idxA = np.zeros(BA3 * P, dtype=np.int16)
        idxB = np.zeros(BB3 * P, dtype=np.int16)
        colid = np.full((NBLK3, P), -1.0, dtype=np.float32)
        cslv = np.zeros((NBLK3, P), dtype=np.float32)
        for half, (idxarr, blk0, off) in enumerate([(idxA, 0, 0), (idxB, BA3, HALF)]):
            mm = hh == half
            gg, cc = ed[mm], colpos[mm]
            pos = np.arange(len(gg))
            idxarr[pos] = (gg - off).astype(np.int16)
            colid[blk0 + pos // P, pos % P] = cc.astype(np.float32)
            cslv[blk0 + pos // P, pos % P] = cfull[gg]
        kpad = np.zeros(KP, dtype=np.int64)
        kpad[:len(kn)] = kn
        zk = np.zeros((KP, D), dtype=np.float32)
        zk[:len(kn)] = zfull[kn]
        Ck = np.zeros((KP, D), dtype=np.float32)
        Ck[:len(kn)] = C16full[kn].astype(np.float32)
        ckv = np.zeros((KP, 1), dtype=np.float32)
        ckv[:len(kn), 0] = np.where(np.abs(cfull[kn]) >= EPS_C, cfull[kn], 0.0)
        indk = np.zeros((KP, 1), dtype=np.float32)
        indk[:len(kn), 0] = indeg[kn]
        maps3.append(dict(
            z16=z16full, zk=zk, Ck=Ck, ck=ckv, indegk=indk,
            wcol=np.full((KP, 1), wv, dtype=np.float32),
            idxA=wrap_idx(idxA), idxB=wrap_idx(idxB),
            colid=np.ascontiguousarray(colid.T),
            csl=np.ascontiguousarray(cslv.T),
        ))
    res3 = runner(nc3, maps3, ["outk"])
    return res3, percore_nodes


# ----------------------------------------------------------------------------
# Entry point + timing plumbing
# ----------------------------------------------------------------------------

TRACE = False
TIME_REPS = 0
LAST_EXEC_TIMES = []
STUB_TIMES = []


def _hw_runner(nc, in_maps, out_names):
    from concourse.bass_utils import run_bass_kernel_spmd
    res = run_bass_kernel_spmd(nc, in_maps, core_ids=list(range(len(in_maps))),
                               trace=TRACE)
    if TIME_REPS:
        LAST_EXEC_TIMES.append(_time_launch(nc, in_maps, TIME_REPS))
        STUB_TIMES.append(_stub_like(nc, in_maps))
    return res.results


def kernel(**inputs):
    """Full (unsharded) inputs -> full output [50000, 128] float32."""
    inputs_np = {k: np.asarray(v) for k, v in inputs.items()}
    del LAST_EXEC_TIMES[:]
    del STUB_TIMES[:]
    out = full_pipeline(inputs_np, _hw_runner)
    return np.ascontiguousarray(out.astype(np.float32))


def _make_sharded(nc, in_maps):
    import jax
    import concourse.mybir as _mybir
    from concourse import bass2jax as b2j
    from jax.experimental.shard_map import shard_map
    from jax.sharding import Mesh, PartitionSpec, NamedSharding

    b2j.install_neuronx_cc_hook()
    n_cores = len(in_maps)
    partition_name = nc.partition_id_tensor.name if nc.partition_id_tensor else None
    in_names, out_names, out_avals, zero_outs = [], [], [], []
    for alloc in nc.m.functions[0].allocations:
        if not isinstance(alloc, _mybir.MemoryLocationSet):
            continue
        name = alloc.memorylocations[0].name
        if alloc.kind == "ExternalInput":
            if name != partition_name:
                in_names.append(name)
        elif alloc.kind == "ExternalOutput":
            shape = tuple(alloc.tensor_shape)
            dtype = _mybir.dt.np(alloc.dtype)
            out_names.append(name)
            out_avals.append(jax.core.ShapedArray(shape, dtype))
            zero_outs.append(np.zeros(shape, dtype))
    n_params = len(in_names)
    all_in = in_names + out_names
    if partition_name is not None:
        all_in = all_in + [partition_name]

    def _body(*args):
        operands = list(args)
        if partition_name is not None:
            operands.append(b2j.partition_id_tensor())
        outs = b2j._bass_exec_p.bind(
            *operands,
            out_avals=tuple(out_avals),
            in_names=tuple(all_in),
            out_names=tuple(out_names),
            lowering_input_output_aliases=(),
            sim_require_finite=True,
            sim_require_nnan=True,
            nc=nc,
        )
        return tuple(outs)

    devices = jax.devices()[:n_cores]
    mesh = Mesh(np.asarray(devices), ("core",))
    spec = PartitionSpec("core")
    in_specs = (spec,) * (n_params + len(out_names))
    out_specs = (spec,) * len(out_names)
    fn = jax.jit(
        shard_map(_body, mesh=mesh, in_specs=in_specs, out_specs=out_specs,
                  check_rep=False),
        keep_unused=True,
    )
    sh = NamedSharding(mesh, spec)
    concat_in = [
        jax.device_put(
            np.concatenate([np.asarray(in_maps[c][nm]) for c in range(n_cores)],
                           axis=0),
            sh,
        )
        for nm in in_names
    ]
    concat_zero = [
        jax.device_put(np.zeros((n_cores * z.shape[0], *z.shape[1:]), z.dtype), sh)
        for z in zero_outs
    ]
    return fn, concat_in, concat_zero


def _time_launch(nc, in_maps, reps):
    import jax, time as _time
    fn, concat_in, concat_zero = _make_sharded(nc, in_maps)
    walls = []
    for _ in range(reps + 1):
        t0 = _time.perf_counter()
        outs = fn(*concat_in, *concat_zero)
        jax.block_until_ready(outs)
        walls.append((_time.perf_counter() - t0) * 1e9)
    return min(walls[1:]) if len(walls) > 1 else walls[0]


def _stub_like(nc_ref, in_maps):
    """Stub kernel with the same ExternalInputs/Outputs as nc_ref: measures
    dispatch + framework + input-binding overhead without the real work."""
    import concourse.mybir as _mybir
    nc = bacc.Bacc("TRN2", target_bir_lowering=False, debug=False)
    outs = []
    for alloc in nc_ref.m.functions[0].allocations:
        if not isinstance(alloc, _mybir.MemoryLocationSet):
            continue
        ml = alloc.memorylocations[0]
        if alloc.kind == "ExternalInput":
            if nc_ref.partition_id_tensor and ml.name == nc_ref.partition_id_tensor.name:
                continue
            nc.dram_tensor(ml.name, list(alloc.tensor_shape), alloc.dtype,
                           kind="ExternalInput")
        elif alloc.kind == "ExternalOutput":
            outs.append(nc.dram_tensor(ml.name, list(alloc.tensor_shape),
                                       alloc.dtype, kind="ExternalOutput"))
    with tile.TileContext(nc) as tc:
        with tc.tile_pool(name="w", bufs=1) as wp:
            for o in outs:
                t = wp.tile([1, 16], o.dtype, tag="t")
                nc.vector.memset(t[:], 0)
                sl = tuple([slice(0, 1)] * (len(o.shape) - 1)
                           + [slice(0, min(16, o.shape[-1]))])
                nc.sync.dma_start(o[sl], t[0:1, 0:min(16, o.shape[-1])])
    nc.compile()
    return _time_launch(nc, in_maps, 3)
